# revision 41
# baseline (speedup 1.0000x reference)
"""BiDAF attention layer on 8 Trainium2 NeuronCores (Bass/Tile).

Math (per batch b):
  t[i,j]  = sum_d (c[i,d]*w_cq[d] + w_q[d]) * q[j,d]   (= cq + sq0[j])
  a       = softmax_j(t)            (biases b_c/b_q/b_cq cancel in softmax)
  c2q     = a @ q
  m[i]    = max_j t[i,j];  sc0[i] = c[i,:]@w_c
  bvec    = softmax_i(m + sc0)      (biases cancel here too)
  q2c     = bvec @ c
  out     = [c | c2q | c*c2q | c*q2c]

Sharding: data-parallel over batch, 4 batches per core, params replicated.

Implementation notes (structural rework of the 135-138us baseline; ~120-123us):
  - c is read from HBM once (f32); the fp16 copy needed for transposes and
    the q2c matvec comes from an SBUF->SBUF SWDGE cast DMA (no extra HBM
    traffic, no engine time). Batch 0 casts straight from DRAM so its
    descgen doesn't serialize the gpsimd queue behind the c load.
  - scores computed twice on PE (t [i,j] for row-max, tT [j,i] so the
    ScalarE exp lands e^T in exactly the c2q lhsT layout), fp16, f32 PSUM.
    (fp8 DoubleRow for the max side was tried and measured SLOWER: the
    [128,2,512] DR matmuls run ~630ns on HW vs 2x213ns for fp16.)
  - the old "+sc0 broadcast" K=1 matmuls (8x512cyc/batch) are gone: sc0 is
    computed once as a [1,CL] row (M=1 matmuls over cT), transposed to
    [P,NT] with 8 tiny PE transposes, and added to the row-maxes on DVE
    before the bvec exp.
  - row maxes are taken two i-tiles at a time from a [P,2,QL] PSUM pair
    (halves the DVE reduce op count).
  - PSUM: one "wide" pool (2 banks x 2 bufs: transpose targets + max pairs)
    and one "narrow" pool (1 bank x 4 bufs: score tiles, c2q, small mms).
  - transpose PSUM is evacuated in 1024-wide ops (1 per q, 2 per c side).
  - emission order matters (engine queues are in-order): the serial
    bvec->q2c->c4 cross-engine chain is emitted AFTER the first c2q half
    so the PE never idles waiting for ebv; for the LAST batch it's
    emitted before mm2 instead, so the c4 chain and stores drain under
    the final matmuls. (A deeper software pipeline hoisting the next
    batch's transposes into the c2q phase measured ~10us slower, as did
    a PE clock-warm-up via dummy transposes.)
  - c*c2q and c*q2c element-wise blocks are split DVE/GpSimd; stores are
    split per-half / per-2-tiles so DMA drains while compute continues.
  - DMA rings: c-in/block0-out on sync(SP), stage-out on scalar(ACT),
    q-in (f32->fp16 cast in flight), c_h cast and c4-out on gpsimd.
  - NOTE: device timing noise between identical runs is +/-10% (shared
    device, DVFS); comparisons used interleaved A/B taking minimums.
"""

import sys

if "/opt/trn_rl_repo" not in sys.path:
    sys.path.insert(0, "/opt/trn_rl_repo")

import numpy as np

import concourse.bass as bass
import concourse.tile as tile
from concourse import bacc, mybir
from concourse.bass import ds, ts
from concourse.masks import make_identity

B, CL, QL, D = 32, 1024, 512, 256
NCORES = 8
BS = B // NCORES  # batches per core
P = 128
F32 = mybir.dt.float32
F16 = mybir.dt.float16

NT = CL // P  # 8 i-tiles
NJ = QL // P  # 4 j-chunks
ND = D // P   # 2 d-chunks
NH = 2        # i-halves for the [j,i]-layout score matmul
IH = CL // NH  # 512
KPH = NT // NH  # i-tiles per half
NPR = NT // 2  # i-tile pairs (for the paired row-max)

Exp = mybir.ActivationFunctionType.Exp
Copy = mybir.ActivationFunctionType.Copy
AxX = mybir.AxisListType.X
Mult = mybir.AluOpType.mult
Add = mybir.AluOpType.add


def build_bass(bs: int = BS):
    nc = bacc.Bacc(None)
    c_d = nc.declare_dram_parameter("c", [bs, CL, D], F32, isOutput=False)
    q_d = nc.declare_dram_parameter("q", [bs, QL, D], F32, isOutput=False)
    wc_d = nc.declare_dram_parameter("wc_cols", [P, ND], F16, isOutput=False)
    wq_d = nc.declare_dram_parameter("wq_cols", [P, ND], F32, isOutput=False)
    wcq_d = nc.declare_dram_parameter("wcq_cols", [P, ND], F32, isOutput=False)
    out_d = nc.declare_dram_parameter("out", [bs, CL, 4 * D], F32, isOutput=True)

    with tile.TileContext(nc) as tc:
        with (
            tc.tile_pool(name="consts", bufs=1) as consts,
            tc.tile_pool(name="io", bufs=2) as io,
            tc.tile_pool(name="ins", bufs=3) as ins,
            tc.tile_pool(name="work", bufs=3) as work,
            tc.tile_pool(name="ps_w", bufs=2, space="PSUM") as ps_w,
            tc.tile_pool(name="ps_n", bufs=4, space="PSUM") as ps_n,
        ):
            ident_h = consts.tile([P, P], F16)
            ones_f = consts.tile([P, P], F32)
            neg_shift = consts.tile([P, 1], F32)
            wc_sb = consts.tile([P, ND], F16)
            wq_sb = consts.tile([P, ND], F32)
            wcq_sb = consts.tile([P, ND], F32)

            def emit_inputs(b):
                # q loaded once, cast f32 -> fp16 in-flight (SWDGE); split in
                # halves so the first transposes start before the full load.
                q_sb = ins.tile([P, NJ, D + 1], F16, tag="q_sb")
                qv = q_d[b].rearrange("(t p) d -> p t d", p=P)
                for h in range(NH):
                    nc.gpsimd.dma_start(
                        out=q_sb[:, ds(h * 2, 2), 0:D], in_=qv[:, ds(h * 2, 2)]
                    )
                nc.vector.memset(q_sb[:, :, D : D + 1], 1.0)
                c_sb = ins.tile([P, NT, D], F32, tag="c_sb")
                cv = c_d[b].rearrange("(t p) d -> p t d", p=P)
                ov = out_d[b].rearrange("(t p) x -> p t x", p=P)
                c_h = ins.tile([P, NT, D], F16, tag="c_h")
                # both c halves load before the block-0 stores hit the SP
                # ring, so the store transfers don't delay the second half
                for h in range(NH):
                    sl = ds(h * KPH, KPH)
                    nc.sync.dma_start(out=c_sb[:, sl], in_=cv[:, sl])
                    # fp16 copy: SWDGE cast. Batch 0 reads DRAM directly so
                    # its descgen doesn't block the gpsimd queue on c_sb.
                    if b == 0:
                        nc.gpsimd.dma_start(out=c_h[:, sl], in_=cv[:, sl])
                    else:
                        nc.gpsimd.dma_start(out=c_h[:, sl], in_=c_sb[:, sl])
                for h in range(NH):
                    sl = ds(h * KPH, KPH)
                    # block 0 of the output is just c (SP ring, from SBUF)
                    nc.sync.dma_start(out=ov[:, sl, 0:D], in_=c_sb[:, sl])
                return c_sb, q_sb, c_h, ov

            # consts first: nothing above them in any engine queue
            nc.scalar.dma_start(out=wc_sb, in_=wc_d[:])
            nc.scalar.dma_start(out=wq_sb, in_=wq_d[:])
            nc.scalar.dma_start(out=wcq_sb, in_=wcq_d[:])
            make_identity(nc, ident_h)
            nc.vector.memset(ones_f, 1.0)
            nc.vector.memset(neg_shift, -2.5)

            pending = [emit_inputs(0)]

            for b in range(bs):
                c_sb, q_sb, c_h, ov = pending.pop(0)

                if b == 0:
                    for nb in (1, 2):
                        if nb < bs:
                            pending.append(emit_inputs(nb))
                elif b + 2 < bs:
                    pending.append(emit_inputs(b + 2))

                # ------------- transpose q -> qT (fp16) -------------
                # qp is only 2KB/partition: take it from the narrow pool so
                # the c-transpose tiles don't wait on the qT evacuation to
                # recycle a wide-pool slot
                qp = ps_n.tile([P, ND, QL], F16, tag="n")
                for dc in range(ND):
                    for jc in range(NJ):
                        nc.tensor.transpose(
                            qp[:, dc, ts(jc, P)], q_sb[:, jc, ts(dc, P)], ident_h
                        )
                qT = work.tile([P, ND, QL], F16, tag="qT")
                nc.scalar.copy(qT, qp)

                # ---- transpose c_h; evacuate as cT (plain) + chatT (affine) ----
                cT = work.tile([P, ND, CL], F16, tag="cT")
                chatT = work.tile([P, ND, CL], F16, tag="chatT")
                for dc in range(ND):
                    cp = ps_w.tile([P, CL], F16, tag="w")
                    for it in range(NT):
                        nc.tensor.transpose(
                            cp[:, ts(it, P)], c_h[:, it, ts(dc, P)], ident_h
                        )
                    nc.vector.tensor_scalar(
                        out=chatT[:, dc],
                        in0=cp,
                        scalar1=wcq_sb[:, dc : dc + 1],
                        scalar2=wq_sb[:, dc : dc + 1],
                        op0=Mult,
                        op1=Add,
                    )
                    nc.scalar.copy(cT[:, dc], cp)

                # ---- phase M: scores ----
                # e^T side: tT [j,i] per (h, jc), exp evacuated to eT
                eTs = []
                for h in range(NH):
                    eT = work.tile([P, NJ, IH], F16, tag="eT")
                    eTs.append(eT)
                    for jc in range(NJ):
                        tp = ps_n.tile([P, IH], F32, tag="n")
                        for dc in range(ND):
                            nc.tensor.matmul(
                                tp,
                                qT[:, dc, ts(jc, P)],
                                chatT[:, dc, ds(h * IH, IH)],
                                start=(dc == 0),
                                stop=(dc == ND - 1),
                            )
                        nc.scalar.activation(eT[:, jc], tp, Exp)

                # row-max side: t [i,j] two i-tiles at a time
                m_all = work.tile([P, NT], F32, tag="m_all")
                for pr in range(NPR):
                    t2 = ps_w.tile([P, 2, QL], F32, tag="w")
                    for s2 in range(2):
                        it = 2 * pr + s2
                        for dc in range(ND):
                            nc.tensor.matmul(
                                t2[:, s2],
                                chatT[:, dc, ts(it, P)],
                                qT[:, dc],
                                start=(dc == 0),
                                stop=(dc == ND - 1),
                            )
                    nc.vector.reduce_max(m_all[:, ds(2 * pr, 2)], t2, AxX)

                # ---- sc0 row [1,CL] via M=1 matmuls; transposed to [P,NT].
                # Emitted AFTER the score matmuls: the cT evacuation drains
                # through ACT's in-order queue behind the previous batch's
                # exps/scales, so putting the cT-dependent matmuls last in
                # phase M removes the measured per-batch PE stall. ----
                sc0_row = work.tile([1, CL], F32, tag="sc0r")
                for h in range(NH):
                    ps_sr = ps_n.tile([1, IH], F32, tag="n")
                    for dc in range(ND):
                        nc.tensor.matmul(
                            ps_sr,
                            wc_sb[:, dc : dc + 1],
                            cT[:, dc, ds(h * IH, IH)],
                            start=(dc == 0),
                            stop=(dc == ND - 1),
                        )
                    nc.vector.tensor_copy(sc0_row[0:1, ds(h * IH, IH)], ps_sr)
                sc_pp = ps_n.tile([P, NT], F32, tag="n")
                for it in range(NT):
                    nc.tensor.transpose(
                        sc_pp[:, it : it + 1],
                        sc0_row[0:1, ts(it, P)],
                        ones_f[0:1, 0:1],
                    )
                sc0_pt = work.tile([P, NT], F32, tag="sc0pt")
                nc.vector.tensor_copy(sc0_pt, sc_pp)

                # ---- bvec numerators: ebv = exp(m + sc0 - 2.5) (fp16) ----
                xm = work.tile([P, NT], F32, tag="xm")
                nc.vector.tensor_add(xm, m_all, sc0_pt)
                ebv_h = work.tile([P, NT], F16, tag="ebvh")
                nc.scalar.activation(ebv_h, xm, Exp, bias=neg_shift[:, 0:1])
                colsum = work.tile([P, 1], F32, tag="colsum")
                nc.vector.reduce_sum(colsum, ebv_h, AxX)

                # ---- c2q matmuls + output blocks 1,2 ----
                stage = io.tile([P, NT, 2 * D], F32, tag="stage")

                def mm2_tile(h, k):
                    it = h * KPH + k
                    po = ps_n.tile([P, D + 1], F32, tag="n")
                    for jc in range(NJ):
                        nc.tensor.matmul(
                            po,
                            eTs[h][:, jc, ts(k, P)],
                            q_sb[:, jc],
                            start=(jc == 0),
                            stop=(jc == NJ - 1),
                        )
                    linv = work.tile([P, 1], F32, tag="linv")
                    nc.vector.reciprocal(linv, po[:, D : D + 1])
                    if k % 2 == 0:
                        nc.scalar.mul(stage[:, it, 0:D], po[:, 0:D], linv)
                        nc.vector.tensor_mul(
                            stage[:, it, D : 2 * D], c_sb[:, it], stage[:, it, 0:D]
                        )
                    else:
                        nc.vector.tensor_scalar_mul(
                            stage[:, it, 0:D], po[:, 0:D], linv
                        )
                        nc.gpsimd.tensor_mul(
                            stage[:, it, D : 2 * D], c_sb[:, it], stage[:, it, 0:D]
                        )

                def emit_q2c_c4():
                    # serial bvec -> q2c cross-engine chain, then c4 blocks
                    ps_tot = ps_n.tile([P, 1], F32, tag="n")
                    nc.tensor.matmul(
                        ps_tot, ones_f, colsum, start=True, stop=True
                    )
                    totinv = work.tile([P, 1], F32, tag="totinv")
                    nc.vector.reciprocal(totinv, ps_tot)
                    ps_q2c = ps_n.tile([1, D], F32, tag="n")
                    for it in range(NT):
                        nc.tensor.matmul(
                            ps_q2c,
                            ebv_h[:, it : it + 1],
                            c_h[:, it],
                            start=(it == 0),
                            stop=(it == NT - 1),
                        )
                    q2c_row = work.tile([1, D], F32, tag="q2cr")
                    nc.vector.tensor_scalar_mul(
                        q2c_row, ps_q2c, totinv[0:1, 0:1]
                    )
                    ps_q2cb = ps_n.tile([P, D], F32, tag="n")
                    nc.tensor.matmul(
                        ps_q2cb, ones_f[0:1, :], q2c_row, start=True, stop=True
                    )
                    q2c_sb = work.tile([P, D], F32, tag="q2csb")
                    nc.scalar.copy(q2c_sb, ps_q2cb)

                    c4st = io.tile([P, NT, D], F32, tag="c4st")
                    for h in range(NH):
                        for k in range(KPH):
                            it = h * KPH + k
                            eng = nc.gpsimd if (k % 2) == 0 else nc.vector
                            eng.tensor_mul(c4st[:, it], c_sb[:, it], q2c_sb)
                        nc.gpsimd.dma_start(
                            out=ov[:, ds(h * KPH, KPH), 3 * D : 4 * D],
                            in_=c4st[:, ds(h * KPH, KPH)],
                        )

                last = b == bs - 1
                if last:
                    # no batch follows: a small PE stall on the ebv wait is
                    # free, and the c4 chain + stores drain under mm2
                    emit_q2c_c4()

                for k in range(KPH):
                    mm2_tile(0, k)
                nc.scalar.dma_start(
                    out=ov[:, 0:KPH, D : 3 * D], in_=stage[:, 0:KPH]
                )

                if not last:
                    # after mm2 h0: PE is in-order, so the cross-engine ebv
                    # latency hides under the h0 matmuls
                    emit_q2c_c4()

                for k in range(KPH):
                    mm2_tile(1, k)
                    if last and k % 2 == 1:
                        nc.scalar.dma_start(
                            out=ov[:, ds(KPH + k - 1, 2), D : 3 * D],
                            in_=stage[:, ds(KPH + k - 1, 2)],
                        )
                if not last:
                    nc.scalar.dma_start(
                        out=ov[:, KPH:NT, D : 3 * D], in_=stage[:, KPH:NT]
                    )

    nc.compile()
    return nc


_NC_CACHE = {}


def _get_nc(bs: int = BS):
    if bs not in _NC_CACHE:
        _NC_CACHE[bs] = build_bass(bs)
    return _NC_CACHE[bs]


def _param_maps(w_c, w_q, w_cq):
    wc_cols = np.ascontiguousarray(
        np.asarray(w_c, np.float32).reshape(ND, P).T.astype(np.float16)
    )
    wq_cols = np.ascontiguousarray(np.asarray(w_q, np.float32).reshape(ND, P).T)
    wcq_cols = np.ascontiguousarray(
        np.asarray(w_cq, np.float32).reshape(ND, P).T
    )
    return wc_cols, wq_cols, wcq_cols


def _run(c, q, w_c, w_q, w_cq, trace=False, **trace_kwargs):
    from concourse.bass_utils import run_bass_kernel_spmd

    c = np.asarray(c, np.float32)
    q = np.asarray(q, np.float32)
    wc_cols, wq_cols, wcq_cols = _param_maps(w_c, w_q, w_cq)

    nc = _get_nc(BS)
    in_maps = []
    for k in range(NCORES):
        in_maps.append(
            {
                "c": np.ascontiguousarray(c[k * BS : (k + 1) * BS]),
                "q": np.ascontiguousarray(q[k * BS : (k + 1) * BS]),
                "wc_cols": wc_cols,
                "wq_cols": wq_cols,
                "wcq_cols": wcq_cols,
            }
        )
    res = None
    last_err = None
    for attempt in range(3):
        try:
            res = run_bass_kernel_spmd(
                nc,
                in_maps,
                core_ids=list(range(NCORES)),
                trace=trace,
                **trace_kwargs,
            )
            break
        except Exception as e:  # transient device wedges clear on retry
            last_err = e
            if "UNRECOVERABLE" not in str(e) and "UNAVAILABLE" not in str(e):
                raise
    if res is None:
        raise last_err
    out = np.concatenate([res.results[k]["out"] for k in range(NCORES)], axis=0)
    return out, res


def kernel(c, q, w_c, b_c, w_q, b_q, w_cq, b_cq):
    # b_c/b_q/b_cq provably cancel in both softmaxes; output doesn't use them.
    out, _ = _run(c, q, w_c, w_q, w_cq)
    return out


# revision 42
# speedup vs baseline: 1.1186x; 1.1186x over previous
"""BiDAF attention layer on 8 Trainium2 NeuronCores (Bass/Tile).

Math (per batch b):
  t[i,j]  = sum_d (c[i,d]*w_cq[d] + w_q[d]) * q[j,d]   (= cq + sq0[j])
  a       = softmax_j(t)            (biases b_c/b_q/b_cq cancel in softmax)
  c2q     = a @ q
  m[i]    = max_j t[i,j];  sc0[i] = c[i,:]@w_c
  bvec    = softmax_i(m + sc0)      (biases cancel here too)
  q2c     = bvec @ c
  out     = [c | c2q | c*c2q | c*q2c]

Sharding: data-parallel over batch, 4 batches per core, params replicated.

Implementation notes (structural rework of the 135-138us baseline; ~120-123us):
  - c is read from HBM once (f32); the fp16 copy needed for transposes and
    the q2c matvec comes from an SBUF->SBUF SWDGE cast DMA (no extra HBM
    traffic, no engine time). Batch 0 casts straight from DRAM so its
    descgen doesn't serialize the gpsimd queue behind the c load.
  - scores computed twice on PE (t [i,j] for row-max, tT [j,i] so the
    ScalarE exp lands e^T in exactly the c2q lhsT layout), fp16, f32 PSUM.
    (fp8 DoubleRow for the max side was tried and measured SLOWER: the
    [128,2,512] DR matmuls run ~630ns on HW vs 2x213ns for fp16.)
  - the old "+sc0 broadcast" K=1 matmuls (8x512cyc/batch) are gone: sc0 is
    computed once as a [1,CL] row (M=1 matmuls over cT), transposed to
    [P,NT] with 8 tiny PE transposes, and added to the row-maxes on DVE
    before the bvec exp.
  - row maxes are taken two i-tiles at a time from a [P,2,QL] PSUM pair
    (halves the DVE reduce op count).
  - PSUM: one "wide" pool (2 banks x 2 bufs: transpose targets + max pairs)
    and one "narrow" pool (1 bank x 4 bufs: score tiles, c2q, small mms).
  - transpose PSUM is evacuated in 1024-wide ops (1 per q, 2 per c side).
  - emission order matters (engine queues are in-order): the serial
    bvec->q2c->c4 cross-engine chain is emitted AFTER the first c2q half
    so the PE never idles waiting for ebv; for the LAST batch it's
    emitted before mm2 instead, so the c4 chain and stores drain under
    the final matmuls. (A deeper software pipeline hoisting the next
    batch's transposes into the c2q phase measured ~10us slower, as did
    a PE clock-warm-up via dummy transposes.)
  - c*c2q and c*q2c element-wise blocks are split DVE/GpSimd; stores are
    split per-half / per-2-tiles so DMA drains while compute continues.
  - DMA rings: c-in/block0-out on sync(SP), stage-out on scalar(ACT),
    q-in (f32->fp16 cast in flight), c_h cast and c4-out on gpsimd.
  - NOTE: device timing noise between identical runs is +/-10% (shared
    device, DVFS); comparisons used interleaved A/B taking minimums.
"""

import sys

if "/opt/trn_rl_repo" not in sys.path:
    sys.path.insert(0, "/opt/trn_rl_repo")

import numpy as np

import concourse.bass as bass
import concourse.tile as tile
from concourse import bacc, mybir
from concourse.bass import ds, ts
from concourse.masks import make_identity

B, CL, QL, D = 32, 1024, 512, 256
NCORES = 8
BS = B // NCORES  # batches per core
P = 128
F32 = mybir.dt.float32
F16 = mybir.dt.float16

NT = CL // P  # 8 i-tiles
NJ = QL // P  # 4 j-chunks
ND = D // P   # 2 d-chunks
NH = 2        # i-halves for the [j,i]-layout score matmul
IH = CL // NH  # 512
KPH = NT // NH  # i-tiles per half
NPR = NT // 2  # i-tile pairs (for the paired row-max)

Exp = mybir.ActivationFunctionType.Exp
Copy = mybir.ActivationFunctionType.Copy
AxX = mybir.AxisListType.X
Mult = mybir.AluOpType.mult
Add = mybir.AluOpType.add


def build_bass(bs: int = BS):
    nc = bacc.Bacc(None)
    c_d = nc.declare_dram_parameter("c", [bs, CL, D], F32, isOutput=False)
    q_d = nc.declare_dram_parameter("q", [bs, QL, D], F32, isOutput=False)
    wc_d = nc.declare_dram_parameter("wc_cols", [P, ND], F16, isOutput=False)
    wq_d = nc.declare_dram_parameter("wq_cols", [P, ND], F32, isOutput=False)
    wcq_d = nc.declare_dram_parameter("wcq_cols", [P, ND], F32, isOutput=False)
    out_d = nc.declare_dram_parameter("out", [bs, CL, 4 * D], F32, isOutput=True)

    with tile.TileContext(nc) as tc:
        with (
            tc.tile_pool(name="consts", bufs=1) as consts,
            tc.tile_pool(name="io", bufs=2) as io,
            tc.tile_pool(name="ins", bufs=3) as ins,
            tc.tile_pool(name="work", bufs=3) as work,
            tc.tile_pool(name="ps_w", bufs=2, space="PSUM") as ps_w,
            tc.tile_pool(name="ps_n", bufs=4, space="PSUM") as ps_n,
        ):
            ident_h = consts.tile([P, P], F16)
            ones_f = consts.tile([P, P], F32)
            neg_shift = consts.tile([P, 1], F32)
            wc_sb = consts.tile([P, ND], F16)
            wq_sb = consts.tile([P, ND], F32)
            wcq_sb = consts.tile([P, ND], F32)

            def emit_inputs(b):
                # q loaded once, cast f32 -> fp16 in-flight (SWDGE); split in
                # halves so the first transposes start before the full load.
                q_sb = ins.tile([P, NJ, D + 1], F16, tag="q_sb")
                qv = q_d[b].rearrange("(t p) d -> p t d", p=P)
                for h in range(NH):
                    nc.gpsimd.dma_start(
                        out=q_sb[:, ds(h * 2, 2), 0:D], in_=qv[:, ds(h * 2, 2)]
                    )
                nc.vector.memset(q_sb[:, :, D : D + 1], 1.0)
                c_sb = ins.tile([P, NT, D], F32, tag="c_sb")
                cv = c_d[b].rearrange("(t p) d -> p t d", p=P)
                ov = out_d[b].rearrange("(t p) x -> p t x", p=P)
                c_h = ins.tile([P, NT, D], F16, tag="c_h")
                # both c halves load before the block-0 stores hit the SP
                # ring, so the store transfers don't delay the second half
                for h in range(NH):
                    sl = ds(h * KPH, KPH)
                    nc.sync.dma_start(out=c_sb[:, sl], in_=cv[:, sl])
                    # fp16 copy: SWDGE cast. Batch 0 reads DRAM directly so
                    # its descgen doesn't block the gpsimd queue on c_sb.
                    if b == 0:
                        nc.gpsimd.dma_start(out=c_h[:, sl], in_=cv[:, sl])
                    else:
                        nc.gpsimd.dma_start(out=c_h[:, sl], in_=c_sb[:, sl])
                for h in range(NH):
                    sl = ds(h * KPH, KPH)
                    # block 0 of the output is just c (SP ring, from SBUF)
                    nc.sync.dma_start(out=ov[:, sl, 0:D], in_=c_sb[:, sl])
                return c_sb, q_sb, c_h, ov

            # consts first: nothing above them in any engine queue
            nc.scalar.dma_start(out=wc_sb, in_=wc_d[:])
            nc.scalar.dma_start(out=wq_sb, in_=wq_d[:])
            nc.scalar.dma_start(out=wcq_sb, in_=wcq_d[:])
            make_identity(nc, ident_h)
            nc.vector.memset(ones_f, 1.0)
            nc.vector.memset(neg_shift, -2.5)

            pending = [emit_inputs(0)]

            for b in range(bs):
                c_sb, q_sb, c_h, ov = pending.pop(0)

                if b == 0:
                    for nb in (1, 2):
                        if nb < bs:
                            pending.append(emit_inputs(nb))
                elif b + 2 < bs:
                    pending.append(emit_inputs(b + 2))

                # ------------- transpose q -> qT (fp16) -------------
                qp = ps_w.tile([P, ND, QL], F16, tag="w")
                for dc in range(ND):
                    for jc in range(NJ):
                        nc.tensor.transpose(
                            qp[:, dc, ts(jc, P)], q_sb[:, jc, ts(dc, P)], ident_h
                        )
                qT = work.tile([P, ND, QL], F16, tag="qT")
                nc.scalar.copy(qT, qp)

                # ---- transpose c_h; evacuate as cT (plain) + chatT (affine) ----
                cT = work.tile([P, ND, CL], F16, tag="cT")
                chatT = work.tile([P, ND, CL], F16, tag="chatT")
                for dc in range(ND):
                    cp = ps_w.tile([P, CL], F16, tag="w")
                    for it in range(NT):
                        nc.tensor.transpose(
                            cp[:, ts(it, P)], c_h[:, it, ts(dc, P)], ident_h
                        )
                    nc.vector.tensor_scalar(
                        out=chatT[:, dc],
                        in0=cp,
                        scalar1=wcq_sb[:, dc : dc + 1],
                        scalar2=wq_sb[:, dc : dc + 1],
                        op0=Mult,
                        op1=Add,
                    )
                    nc.scalar.copy(cT[:, dc], cp)

                # ---- phase M: scores ----
                # e^T side: tT [j,i] per (h, jc), exp evacuated to eT
                eTs = []
                for h in range(NH):
                    eT = work.tile([P, NJ, IH], F16, tag="eT")
                    eTs.append(eT)
                    for jc in range(NJ):
                        tp = ps_n.tile([P, IH], F32, tag="n")
                        for dc in range(ND):
                            nc.tensor.matmul(
                                tp,
                                qT[:, dc, ts(jc, P)],
                                chatT[:, dc, ds(h * IH, IH)],
                                start=(dc == 0),
                                stop=(dc == ND - 1),
                            )
                        nc.scalar.activation(eT[:, jc], tp, Exp)

                # row-max side: t [i,j] two i-tiles at a time
                m_all = work.tile([P, NT], F32, tag="m_all")
                for pr in range(NPR):
                    t2 = ps_w.tile([P, 2, QL], F32, tag="w")
                    for s2 in range(2):
                        it = 2 * pr + s2
                        for dc in range(ND):
                            nc.tensor.matmul(
                                t2[:, s2],
                                chatT[:, dc, ts(it, P)],
                                qT[:, dc],
                                start=(dc == 0),
                                stop=(dc == ND - 1),
                            )
                    nc.vector.reduce_max(m_all[:, ds(2 * pr, 2)], t2, AxX)

                # ---- sc0 row [1,CL] via M=1 matmuls; transposed to [P,NT].
                # Emitted AFTER the score matmuls: the cT evacuation drains
                # through ACT's in-order queue behind the previous batch's
                # exps/scales, so putting the cT-dependent matmuls last in
                # phase M removes the measured per-batch PE stall. ----
                sc0_row = work.tile([1, CL], F32, tag="sc0r")
                for h in range(NH):
                    ps_sr = ps_n.tile([1, IH], F32, tag="n")
                    for dc in range(ND):
                        nc.tensor.matmul(
                            ps_sr,
                            wc_sb[:, dc : dc + 1],
                            cT[:, dc, ds(h * IH, IH)],
                            start=(dc == 0),
                            stop=(dc == ND - 1),
                        )
                    nc.vector.tensor_copy(sc0_row[0:1, ds(h * IH, IH)], ps_sr)
                sc_pp = ps_n.tile([P, NT], F32, tag="n")
                for it in range(NT):
                    nc.tensor.transpose(
                        sc_pp[:, it : it + 1],
                        sc0_row[0:1, ts(it, P)],
                        ones_f[0:1, 0:1],
                    )
                sc0_pt = work.tile([P, NT], F32, tag="sc0pt")
                nc.vector.tensor_copy(sc0_pt, sc_pp)

                # ---- bvec numerators: ebv = exp(m + sc0 - 2.5) (fp16) ----
                xm = work.tile([P, NT], F32, tag="xm")
                nc.vector.tensor_add(xm, m_all, sc0_pt)
                ebv_h = work.tile([P, NT], F16, tag="ebvh")
                nc.scalar.activation(ebv_h, xm, Exp, bias=neg_shift[:, 0:1])
                colsum = work.tile([P, 1], F32, tag="colsum")
                nc.vector.reduce_sum(colsum, ebv_h, AxX)

                # ---- c2q matmuls + output blocks 1,2 ----
                stage = io.tile([P, NT, 2 * D], F32, tag="stage")

                def mm2_tile(h, k):
                    it = h * KPH + k
                    po = ps_n.tile([P, D + 1], F32, tag="n")
                    for jc in range(NJ):
                        nc.tensor.matmul(
                            po,
                            eTs[h][:, jc, ts(k, P)],
                            q_sb[:, jc],
                            start=(jc == 0),
                            stop=(jc == NJ - 1),
                        )
                    linv = work.tile([P, 1], F32, tag="linv")
                    nc.vector.reciprocal(linv, po[:, D : D + 1])
                    if k % 2 == 0:
                        nc.scalar.mul(stage[:, it, 0:D], po[:, 0:D], linv)
                        nc.vector.tensor_mul(
                            stage[:, it, D : 2 * D], c_sb[:, it], stage[:, it, 0:D]
                        )
                    else:
                        nc.vector.tensor_scalar_mul(
                            stage[:, it, 0:D], po[:, 0:D], linv
                        )
                        nc.gpsimd.tensor_mul(
                            stage[:, it, D : 2 * D], c_sb[:, it], stage[:, it, 0:D]
                        )

                def emit_q2c_c4():
                    # serial bvec -> q2c cross-engine chain, then c4 blocks
                    ps_tot = ps_n.tile([P, 1], F32, tag="n")
                    nc.tensor.matmul(
                        ps_tot, ones_f, colsum, start=True, stop=True
                    )
                    totinv = work.tile([P, 1], F32, tag="totinv")
                    nc.vector.reciprocal(totinv, ps_tot)
                    ps_q2c = ps_n.tile([1, D], F32, tag="n")
                    for it in range(NT):
                        nc.tensor.matmul(
                            ps_q2c,
                            ebv_h[:, it : it + 1],
                            c_h[:, it],
                            start=(it == 0),
                            stop=(it == NT - 1),
                        )
                    q2c_row = work.tile([1, D], F32, tag="q2cr")
                    nc.vector.tensor_scalar_mul(
                        q2c_row, ps_q2c, totinv[0:1, 0:1]
                    )
                    ps_q2cb = ps_n.tile([P, D], F32, tag="n")
                    nc.tensor.matmul(
                        ps_q2cb, ones_f[0:1, :], q2c_row, start=True, stop=True
                    )
                    q2c_sb = work.tile([P, D], F32, tag="q2csb")
                    nc.scalar.copy(q2c_sb, ps_q2cb)

                    c4st = io.tile([P, NT, D], F32, tag="c4st")
                    for h in range(NH):
                        for k in range(KPH):
                            it = h * KPH + k
                            eng = nc.gpsimd if (k % 2) == 0 else nc.vector
                            eng.tensor_mul(c4st[:, it], c_sb[:, it], q2c_sb)
                        nc.gpsimd.dma_start(
                            out=ov[:, ds(h * KPH, KPH), 3 * D : 4 * D],
                            in_=c4st[:, ds(h * KPH, KPH)],
                        )

                last = b == bs - 1
                if last:
                    # no batch follows: a small PE stall on the ebv wait is
                    # free, and the c4 chain + stores drain under mm2
                    emit_q2c_c4()

                for k in range(KPH):
                    mm2_tile(0, k)
                nc.scalar.dma_start(
                    out=ov[:, 0:KPH, D : 3 * D], in_=stage[:, 0:KPH]
                )

                if not last:
                    # after mm2 h0: PE is in-order, so the cross-engine ebv
                    # latency hides under the h0 matmuls
                    emit_q2c_c4()

                for k in range(KPH):
                    mm2_tile(1, k)
                    if last and k % 2 == 1:
                        nc.scalar.dma_start(
                            out=ov[:, ds(KPH + k - 1, 2), D : 3 * D],
                            in_=stage[:, ds(KPH + k - 1, 2)],
                        )
                if not last:
                    nc.scalar.dma_start(
                        out=ov[:, KPH:NT, D : 3 * D], in_=stage[:, KPH:NT]
                    )

    nc.compile()
    return nc


_NC_CACHE = {}


def _get_nc(bs: int = BS):
    if bs not in _NC_CACHE:
        _NC_CACHE[bs] = build_bass(bs)
    return _NC_CACHE[bs]


def _param_maps(w_c, w_q, w_cq):
    wc_cols = np.ascontiguousarray(
        np.asarray(w_c, np.float32).reshape(ND, P).T.astype(np.float16)
    )
    wq_cols = np.ascontiguousarray(np.asarray(w_q, np.float32).reshape(ND, P).T)
    wcq_cols = np.ascontiguousarray(
        np.asarray(w_cq, np.float32).reshape(ND, P).T
    )
    return wc_cols, wq_cols, wcq_cols


def _run(c, q, w_c, w_q, w_cq, trace=False, **trace_kwargs):
    from concourse.bass_utils import run_bass_kernel_spmd

    c = np.asarray(c, np.float32)
    q = np.asarray(q, np.float32)
    wc_cols, wq_cols, wcq_cols = _param_maps(w_c, w_q, w_cq)

    nc = _get_nc(BS)
    in_maps = []
    for k in range(NCORES):
        in_maps.append(
            {
                "c": np.ascontiguousarray(c[k * BS : (k + 1) * BS]),
                "q": np.ascontiguousarray(q[k * BS : (k + 1) * BS]),
                "wc_cols": wc_cols,
                "wq_cols": wq_cols,
                "wcq_cols": wcq_cols,
            }
        )
    res = None
    last_err = None
    for attempt in range(3):
        try:
            res = run_bass_kernel_spmd(
                nc,
                in_maps,
                core_ids=list(range(NCORES)),
                trace=trace,
                **trace_kwargs,
            )
            break
        except Exception as e:  # transient device wedges clear on retry
            last_err = e
            if "UNRECOVERABLE" not in str(e) and "UNAVAILABLE" not in str(e):
                raise
    if res is None:
        raise last_err
    out = np.concatenate([res.results[k]["out"] for k in range(NCORES)], axis=0)
    return out, res


def kernel(c, q, w_c, b_c, w_q, b_q, w_cq, b_cq):
    # b_c/b_q/b_cq provably cancel in both softmaxes; output doesn't use them.
    out, _ = _run(c, q, w_c, w_q, w_cq)
    return out
